# revision 8
# baseline (speedup 1.0000x reference)
"""Trainium2 Bass kernel for a 4-layer dense MLP (H=8192), batch=1.

Tensor-parallel over 8 NeuronCores. v4: fp8 weights, 4x column-tiled PE,
single-collective sharding (col / row+AllReduce / col / row).

  - Hidden-layer weights stream as fp8 e3m4 (x512 scale, de-scaled in the
    sigmoid's scale parameter): 8.4 MB per core per layer, all on the sync
    (SP) HWDGE ring so no weight DMA ever queues behind a dependency-waiting
    instruction. Activations stay fp16 (mixed-dtype matmul); end-to-end error
    vs the f32 reference is ~2e-3 max-rel.

  - The ncfw rendezvous (~58us, one-time, runs inside the first collective)
    dominates any multi-collective schedule, so the layer sharding is chosen
    to need exactly ONE real collective: layer 2 is column-sharded (its input
    is replicated), layer 3 is ROW-sharded -- each core contracts its own
    layer-2 output shard, no gather needed -- producing a full-width [8192]
    partial that one fp16 AllReduce sums (per-core bias pre-folded as b/8).
    Layer 4 is column-sharded from the reduced activation, and the output
    layer is row-sharded with host-side partial summing. A warmup AllGather
    on garbage data fires as the very first gpsimd instruction so the
    rendezvous completes while weights stream and layers 1-3 compute.

  - Matvecs run as 4 concurrent column-tiled matmul streams
    (tile_position=(0,32j)) so the PE consumes 4x128 fp8 weights per cycle;
    partials land on PSUM partitions {0,32,64,96}, one activation (or DVE
    copy, for layer 3's pre-reduce partials) covers all four rows, and
    strided 4-row gather DMAs flatten them.

  - Layer 1 folds its bias into an 11th weight row ([x;s;1] @ [W_in;b0]) and
    runs 4 column-tiled passes of 2048 cols bounced through DRAM into the
    [128, 64] piece-major layout; layer 2's group g contracts pass g's chunks
    as soon as they land.
"""

import numpy as np

H = 8192
D = 11  # input layer size (4 + 6) + folded bias row
OUT = 8
NCORES = 8
SH = H // NCORES  # 1024 columns per core
HF = 512  # half-width
CT = 128  # column-tile width (4 tiles per half)
KC = 64  # contraction chunks of 128 rows
GC = 16  # chunks per DMA group (1 MiB per DMA at fp8)
G = KC // GC  # 4 groups per half
WBUFS = 18  # in-flight weight DMA buffers (18 MiB SBUF)
WSCALE = 512.0  # fp8 weight pre-scale (power of 2; e3m4 normal range)
L1P = 2048  # layer-1 pass width (4 passes)

LAST_RESULTS = None
_CACHE = {}


def _perm_piece():
    """Layer-2 input layout: a_sb[p, k] = a1[(k//8)*1024 + p*8 + (k%8)].
    Returns rows[k, p] = global row index feeding chunk k, partition p."""
    k = np.arange(KC)[:, None]
    p = np.arange(128)[None, :]
    return (k // 8) * 1024 + p * 8 + (k % 8)


def _perm_l4():
    """Layer-4 input layout: a_sb4[p, k] = a3[p*64 + k] (contiguous
    per-partition lines straight out of the AllReduce result)."""
    k = np.arange(KC)[:, None]
    p = np.arange(128)[None, :]
    return p * 64 + k


def _build_nc():
    import concourse.bacc as bacc
    import concourse.mybir as mybir
    import concourse.tile as tile

    f16 = mybir.dt.float16
    f32 = mybir.dt.float32
    f8 = mybir.dt.float8e3
    SIG = mybir.ActivationFunctionType.Sigmoid
    RG = [list(range(NCORES))]
    INV = 1.0 / WSCALE

    nc = bacc.Bacc(
        "TRN2", target_bir_lowering=False, debug=False, num_devices=NCORES
    )

    x_d = nc.dram_tensor("x_cat", [D, 1], f16, kind="ExternalInput")
    win_d = nc.dram_tensor("w_in", [D, H], f16, kind="ExternalInput")
    whh_d = nc.dram_tensor("w_hh", [3, 8, 128, GC * HF], f8, kind="ExternalInput")
    wout_d = nc.dram_tensor("w_out", [128, 8 * OUT], f16, kind="ExternalInput")
    bias_d = nc.dram_tensor("bias", [1, 2 * SH + H], f16, kind="ExternalInput")
    out_d = nc.dram_tensor("out_partial", [1, OUT], f32, kind="ExternalOutput")

    with tile.TileContext(nc) as tc:
        with (
            tc.tile_pool(name="const", bufs=1) as cp,
            tc.tile_pool(name="wpool", bufs=WBUFS) as wp,
            tc.tile_pool(name="apool", bufs=2) as ap,
            tc.tile_pool(name="pspool", bufs=2, space="PSUM") as pp,
            tc.tile_pool(name="dpool", bufs=2, space="DRAM") as dp,
        ):
            # Warmup collective FIRST on the gpsimd/CC queue: starts the
            # one-time ncfw rendezvous ASAP (payload is garbage, ignored).
            warm_in = dp.tile([1, 16], f16, tag="warmin")
            warm_out = dp.tile([8, 16], f16, tag="warmout")
            nc.gpsimd.collective_compute(
                "AllGather",
                mybir.AluOpType.bypass,
                replica_groups=RG,
                ins=[warm_in.opt()],
                outs=[warm_out.opt()],
            )

            one_sb = cp.tile([1, 1], f16)
            nc.gpsimd.memset(one_sb[:], 1.0)

            x_sb = cp.tile([D, 1], f16)
            nc.scalar.dma_start(x_sb[:], x_d[:])
            win_sb = cp.tile([D, H], f16)
            nc.scalar.dma_start(win_sb[:], win_d[:])
            bias_sb = cp.tile([1, 2 * SH + H], f16)
            nc.scalar.dma_start(bias_sb[:], bias_d[:])
            woutA_sb = cp.tile([64, 8 * OUT], f16)
            nc.scalar.dma_start(woutA_sb[:], wout_d[0:64, :])
            woutB_sb = cp.tile([64, 8 * OUT], f16)
            nc.scalar.dma_start(woutB_sb[:], wout_d[64:128, :])

            # ---- Layer 1, replicated: 4 col-tiled passes of 2048 cols,
            # bounced into the [128, 64] piece-major layout for layer 2 ----
            a1_d = dp.tile([4, L1P], f16, tag="a1")
            a_sb = ap.tile([128, KC], f16, tag="a")
            for p in range(4):
                psA = pp.tile([128, HF], f32, tag=f"psA{p % 2}", bufs=1)
                for j in range(4):
                    nc.tensor.matmul(
                        psA[32 * j : 32 * j + 1, :],
                        x_sb[:],
                        win_sb[:, p * L1P + j * HF : p * L1P + (j + 1) * HF],
                        start=True,
                        stop=True,
                        tile_position=(0, 32 * j),
                    )
                act1 = ap.tile([128, HF], f16, tag=f"act1_{p % 2}")
                nc.scalar.activation(act1[0:97, :], psA[0:97, :], SIG)
                nc.scalar.dma_start(a1_d[p], act1[0:97:32, :])
                nc.scalar.dma_start(
                    a_sb[:, 16 * p : 16 * p + 16],
                    a1_d[p].rearrange("(h p m) -> p h m", h=2, p=128),
                )

            # ---- Layer 2: column-sharded, two 512-col halves, 4x col-tiled;
            # own shard lands in sc2 -> a2_sb [128, 8], no collective ----
            sc2 = dp.tile([1, SH], f16, tag="sc2")
            for hf in range(2):
                ps = pp.tile([128, CT], f32, tag=f"ps{hf}")
                for g in range(G):
                    wt = wp.tile([128, GC * HF], f8, tag="w")
                    nc.sync.dma_start(wt[:], whh_d[0, hf * G + g])
                    for c in range(GC):
                        k = g * GC + c
                        for j in range(4):
                            nc.tensor.matmul(
                                ps[32 * j : 32 * j + 1, :],
                                a_sb[:, k : k + 1],
                                wt[:, c * HF + CT * j : c * HF + CT * (j + 1)],
                                start=(k == 0),
                                stop=False,
                                tile_position=(0, 32 * j),
                            )
                for j in range(4):
                    nc.tensor.matmul(
                        ps[32 * j : 32 * j + 1, :],
                        one_sb[:],
                        bias_sb[:, hf * HF + CT * j : hf * HF + CT * (j + 1)],
                        start=False,
                        stop=True,
                        tile_position=(0, 32 * j),
                    )
                act2 = ap.tile([128, CT], f16, tag=f"act2_{hf}")
                nc.scalar.activation(act2[0:97, :], ps[0:97, :], SIG, scale=INV)
                nc.scalar.dma_start(sc2[:, hf * HF : (hf + 1) * HF], act2[0:97:32, :])
            a2_sb = ap.tile([128, 8], f16, tag="a2")
            nc.gpsimd.dma_start(
                a2_sb[:], sc2.rearrange("one (p k) -> (one p) k", p=128)
            )

            # ---- Layer 3: ROW-sharded. 4 psum tiles of 2048 cols, partials
            # (with b/8 folded) to ccin; one fp16 AllReduce sums them ----
            ccin = dp.tile([1, H], f16, tag="ccin")
            aout = dp.tile([1, H], f16, tag="aout")
            for t in range(4):
                psR = pp.tile([128, HF], f32, tag=f"psA{t % 2}", bufs=1, name="psR")
                for h2 in range(2):
                    wt = wp.tile([128, GC * HF], f8, tag="w", name="wt3")
                    nc.sync.dma_start(wt[:], whh_d[1, t * 2 + h2])
                    for k in range(8):
                        for j2 in range(2):
                            j = 2 * h2 + j2
                            nc.tensor.matmul(
                                psR[32 * j : 32 * j + 1, :],
                                a2_sb[:, k : k + 1],
                                wt[:, k * 1024 + j2 * HF : k * 1024 + (j2 + 1) * HF],
                                start=(k == 0),
                                stop=False,
                                tile_position=(0, 32 * j),
                            )
                for j in range(4):
                    bo = 2 * SH + t * L1P + HF * j
                    nc.tensor.matmul(
                        psR[32 * j : 32 * j + 1, :],
                        one_sb[:],
                        bias_sb[:, bo : bo + HF],
                        start=False,
                        stop=True,
                        tile_position=(0, 32 * j),
                    )
                part = ap.tile([128, HF], f16, tag=f"part{t % 2}")
                nc.vector.tensor_copy(part[0:97, :], psR[0:97, :])
                nc.scalar.dma_start(
                    ccin[:, t * L1P : (t + 1) * L1P], part[0:97:32, :]
                )
            nc.gpsimd.collective_compute(
                "AllReduce",
                mybir.AluOpType.add,
                replica_groups=RG,
                ins=[ccin.opt()],
                outs=[aout.opt()],
            )
            # reduced z3 (x512, bias included) -> sigmoid -> L4 input layout
            z_sb = ap.tile([128, KC], f16, tag="z")
            nc.gpsimd.dma_start(
                z_sb[:], aout.rearrange("one (p k) -> (one p) k", p=128)
            )
            a_sb4 = ap.tile([128, KC], f16, tag="a", name="a_sb4")
            nc.scalar.activation(a_sb4[:], z_sb[:], SIG, scale=INV)

            # ---- Layer 4: column-sharded, two halves; output stage fused ----
            pso = pp.tile([1, OUT], f32, tag="psO", bufs=1)
            for hf in range(2):
                ps = pp.tile([128, CT], f32, tag=f"ps{hf}", name="ps4")
                for g in range(G):
                    wt = wp.tile([128, GC * HF], f8, tag="w", name="wt4")
                    nc.sync.dma_start(wt[:], whh_d[2, hf * G + g])
                    for c in range(GC):
                        k = g * GC + c
                        for j in range(4):
                            nc.tensor.matmul(
                                ps[32 * j : 32 * j + 1, :],
                                a_sb4[:, k : k + 1],
                                wt[:, c * HF + CT * j : c * HF + CT * (j + 1)],
                                start=(k == 0),
                                stop=False,
                                tile_position=(0, 32 * j),
                            )
                for j in range(4):
                    bo = SH + hf * HF + CT * j
                    nc.tensor.matmul(
                        ps[32 * j : 32 * j + 1, :],
                        one_sb[:],
                        bias_sb[:, bo : bo + CT],
                        start=False,
                        stop=True,
                        tile_position=(0, 32 * j),
                    )
                act4 = ap.tile([128, CT], f16, tag=f"act4_{hf}")
                nc.scalar.activation(act4[0:97, :], ps[0:97, :], SIG, scale=INV)
                a2o = ap.tile([64, OUT], f16, tag=f"a2o{hf}")
                nc.gpsimd.dma_start(a2o[:], act4[0:97:32, :])
                wout_sb = woutA_sb if hf == 0 else woutB_sb
                for k in range(8):
                    nc.tensor.matmul(
                        pso[:],
                        a2o[:, k : k + 1],
                        wout_sb[:, k * OUT : (k + 1) * OUT],
                        start=(hf == 0 and k == 0),
                        stop=(hf == 1 and k == 7),
                    )

            res_sb = ap.tile([1, OUT], f32, tag="res")
            nc.vector.tensor_copy(res_sb[:], pso[:])
            nc.scalar.dma_start(out_d[:], res_sb[:])

    nc.compile()
    return nc


def _prep_inputs(x, s, W_in, W_hh, W_out, b):
    """Shard + quantize + lay out the inputs for each of the 8 cores."""
    import ml_dtypes

    f16 = np.float16
    f8 = ml_dtypes.float8_e3m4
    b_np = np.asarray(b)
    x_cat = np.concatenate(
        [np.asarray(x), np.asarray(s), np.ones(1, np.float32)]
    ).astype(f16)
    x_cat = np.ascontiguousarray(x_cat.reshape(D, 1))
    Whh8 = (np.asarray(W_hh) * WSCALE).astype(f8)  # [3, 8192, 8192]
    Win16 = np.ascontiguousarray(
        np.vstack([np.asarray(W_in), b_np[0:1]]).astype(f16)
    )  # [11, 8192] with bias row folded in
    Wout16 = np.asarray(W_out).astype(f16)  # [8192, 8]
    b2_sc = (b_np[1] * WSCALE).astype(f16)  # layer-2 bias, pre-scaled
    b3_part = (b_np[2] * WSCALE / 8.0).astype(f16)  # per-core share, summed by AR
    b4_sc = (b_np[3] * WSCALE).astype(f16)

    perm2, perm4 = _perm_piece(), _perm_l4()

    in_maps = []
    for c in range(NCORES):
        cs, ce = c * SH, (c + 1) * SH
        whh_c = np.empty((3, 8, 128, GC * HF), f8)
        # L2: column shard, piece-major contraction, halves x groups
        wperm = Whh8[0][:, cs:ce][perm2]  # [64, 128, 1024]
        for hf in range(2):
            arr = wperm[:, :, hf * HF : (hf + 1) * HF]  # [64, 128, 512]
            grp = arr.reshape(G, GC, 128, HF).transpose(0, 2, 1, 3)
            whh_c[0, hf * G : (hf + 1) * G] = grp.reshape(G, 128, GC * HF)
        # L3: row shard [1024, 8192] -> [128 p, 8 k, 8192 cols], groups of
        # 1024 cols: wt[p, k*1024 + u] = wrow[8p + k, 2048t + 1024h2 + u]
        w3 = Whh8[1][cs:ce, :].reshape(128, 8, H)
        for t in range(4):
            for h2 in range(2):
                sl = w3[:, :, t * L1P + h2 * 1024 : t * L1P + (h2 + 1) * 1024]
                whh_c[1, t * 2 + h2] = np.ascontiguousarray(sl).reshape(128, 8192)
        # L4: column shard, a3[64p + k] contraction layout
        wperm4 = Whh8[2][:, cs:ce][perm4]  # [64, 128, 1024]
        for hf in range(2):
            arr = wperm4[:, :, hf * HF : (hf + 1) * HF]
            grp = arr.reshape(G, GC, 128, HF).transpose(0, 2, 1, 3)
            whh_c[2, hf * G : (hf + 1) * G] = grp.reshape(G, 128, GC * HF)
        wout_c = np.ascontiguousarray(Wout16[cs:ce, :].reshape(128, 8 * OUT))
        bias_c = np.concatenate([b2_sc[cs:ce], b4_sc[cs:ce], b3_part])
        in_maps.append(
            {
                "x_cat": x_cat,
                "w_in": Win16,
                "w_hh": np.ascontiguousarray(whh_c),
                "w_out": wout_c,
                "bias": np.ascontiguousarray(bias_c.reshape(1, 2 * SH + H)),
            }
        )
    return in_maps


def kernel(**inputs):
    global LAST_RESULTS
    import os

    from concourse import bass_utils

    if "nc" not in _CACHE:
        _CACHE["nc"] = _build_nc()
    nc = _CACHE["nc"]

    in_maps = _prep_inputs(**inputs)
    trace = bool(int(os.environ.get("BASS_TRACE_KERNEL", "0")))
    res = bass_utils.run_bass_kernel_spmd(
        nc, in_maps, core_ids=list(range(NCORES)), trace=trace
    )
    LAST_RESULTS = res
    partials = np.stack([r["out_partial"][0] for r in res.results])  # [8, 8]
    return partials.sum(axis=0).astype(np.float32)
